# revision 13
# baseline (speedup 1.0000x reference)
"""GQA attention kernel for 8 TRN2 NeuronCores — sequence-split edition.

Problem: x[4,2048,1024], 16 Q heads / 4 KV heads, head_dim 64 (torch-Linear
style projections, softmax(QK^T/8)V, output projection + bias).

Sharding: core c handles (batch b = c//2, query-half qh = c%2): it computes
the FULL output rows for its 1024 query tokens (all 16 heads + o-proj), so
there is NO collective — each core DMAs its own [1024, 1024] f32 slab out.
K/V are computed for all 2048 keys on both cores of a pair (cheap).

The host permutes each core's token order so its own queries come first
(attention is key-order invariant), which keeps the SPMD program uniform.
Q-head order is permuted on the host so each head's 64 q-dims sit at the
same partition offset as its KV head's k-dims (QK lhsT/rhs share a base
partition): device q block j (0..7), offset o in {0,64} holds head
(kv = 2*(j//4) + o//64, g = j%4); wo^T rows are permuted identically.

Inside: q^T/k^T in [dim, token] layout so QK needs no transposes; S^T tiles
[keys=128, queries=512] are exp'd on ACT straight from PSUM; V is augmented
with 64 ones columns so the AV matmul also produces the softmax denominators
on partitions 64:128; normalization = reciprocal_approx_fast (single fast
custom-DVE op) + one DVE multiply writing hid^T in bf16. O-proj + bias-add +
output DMA stream per 512-query block, overlapped with the next block's
attention (ACT exp is the critical engine; everything else hides under it).
"""

import sys
import numpy as np
from contextlib import ExitStack

sys.path.insert(0, "/opt/trn_rl_repo")

import ml_dtypes

from concourse import bass, tile, mybir


# ---------------------------------------------------------------------------
# This walrus build encodes at most 1-2 sync waits per instruction; the stock
# TileContext tail drain packs one wait per live proc onto a single Drain and
# fails codegen ("Too many sync wait commands"). Spread the waits over SP nop
# carriers instead.
def _patched_drain_and_barrier(self, tick_clock, wait_clock):
    from concourse.vector_clock import ScopedClock, VectorClock

    nc = self.nc
    gc = tick_clock.global_clock
    n = len(gc)
    for proc in range(n):
        t = gc[proc]
        if t <= 0:
            continue
        carrier = nc.sync.nop(nofuse=True)
        req = VectorClock([t if i == proc else 0 for i in range(n)])
        wait_clock.add_sem_waits(carrier.ins, ScopedClock({None: req}))
    nc.sync.drain()
    nc.all_engine_barrier()
    assert self.sems is not None
    popped = nc._tile_sem_poison_stack.pop()
    assert popped is self._sem_poison
    nc.clear_and_free_semaphores(list(self.sems.allocated().values()))
    nc.all_engine_barrier()


tile.TileContext._drain_and_barrier = _patched_drain_and_barrier


def _split_excess_waits(nc, max_waits=1):
    """Hoist all but one sync wait per instruction onto dedicated
    EventSemaphore carriers placed immediately before it on the same engine
    (same blocking semantics, one wait per encoded instruction)."""
    n_new = 0
    for bb in nc.main_func.blocks:
        il = list(bb.instructions)
        out = []
        changed = False
        for ins in il:
            si = ins.sync_info
            if si is not None:
                w = list(si.on_wait)
                if len(w) > max_waits:
                    for extra in w[max_waits:]:
                        ev = mybir.InstEventSemaphore(
                            name=f"{ins.name}-wsp{n_new}", engine=ins.engine)
                        n_new += 1
                        ev.sync_info = type(si)(on_wait=[extra], on_update=[])
                        nc.register_instruction(ev, overwrite=True)
                        out.append(ev)
                    si.on_wait = w[:max_waits]
                    changed = True
            out.append(ins)
        if changed:
            bb.instructions = out
# ---------------------------------------------------------------------------

B, N, D = 4, 2048, 1024
DH = 64          # head dim
NQ = 1024        # queries per core
NCORES = 8
P = 128
SCALE = DH ** -0.5
BF16 = mybir.dt.bfloat16
F32 = mybir.dt.float32

NKB = N // P     # 16 key blocks of 128
NKC = D // P     # 8 contraction chunks of 128
KVD = 256        # total kv dims
VW = 260         # v chunk width per key block: 4 x [64 v | 1 one]


def build_nc(st_bufs=2, av_bufs=2, pt_bufs=4):
    nc = bass.Bass(target_bir_lowering=False, debug=False, num_devices=NCORES)

    xt = nc.declare_dram_parameter("xt", [D, N], BF16, isOutput=False)
    wqt = nc.declare_dram_parameter("wqt", [D, D], BF16, isOutput=False)
    wkt = nc.declare_dram_parameter("wkt", [D, KVD], BF16, isOutput=False)
    wvt = nc.declare_dram_parameter("wvt", [D, KVD], BF16, isOutput=False)
    wot = nc.declare_dram_parameter("wot", [D, D], BF16, isOutput=False)
    bo_in = nc.declare_dram_parameter("bo_in", [1, D], F32, isOutput=False)
    out_p = nc.declare_dram_parameter("out_p", [NQ, D], F32, isOutput=True)

    with tile.TileContext(nc) as tc, ExitStack() as ctx:
        const = ctx.enter_context(tc.tile_pool(name="const", bufs=1))
        work = ctx.enter_context(tc.tile_pool(name="work", bufs=1))
        # one shared 4-buf PSUM pool for proj/o-proj/AV tiles (deep av
        # rotation so block N+4's wait is always satisfied) + 2x 2-bank st
        ppool = ctx.enter_context(tc.tile_pool(name="ppool", bufs=4, space="PSUM"))
        stpool = ctx.enter_context(tc.tile_pool(name="stp", bufs=st_bufs, space="PSUM"))
        ptpool = ctx.enter_context(tc.tile_pool(name="ptp", bufs=pt_bufs))
        smallp = ctx.enter_context(tc.tile_pool(name="smallp", bufs=3))
        outp = ctx.enter_context(tc.tile_pool(name="outp", bufs=3))

        # ---- load inputs (K weights + x first: K-proj unblocks earliest) --
        xt_sb = const.tile([P, NKC * N], BF16)
        wkt_sb = const.tile([P, NKC * KVD], BF16)
        wvt_sb = const.tile([P, NKC * KVD], BF16)
        wqt_sb = const.tile([P, NKC * D], BF16)
        wot_sb = const.tile([P, NKC * D], BF16)
        bo_row = const.tile([1, D], F32)
        for kc in range(NKC):
            nc.sync.dma_start(out=wkt_sb[:, kc * KVD:(kc + 1) * KVD],
                              in_=wkt[kc * P:(kc + 1) * P, :])
        for kc in range(NKC):
            # x on the gpsimd-triggered queue, in parallel with the weight
            # stream on the sync queue
            nc.gpsimd.dma_start(out=xt_sb[:, kc * N:(kc + 1) * N],
                                in_=xt[kc * P:(kc + 1) * P, :])
        for kc in range(NKC):
            nc.sync.dma_start(out=wvt_sb[:, kc * KVD:(kc + 1) * KVD],
                              in_=wvt[kc * P:(kc + 1) * P, :])
        for kc in range(NKC):
            nc.sync.dma_start(out=wqt_sb[:, kc * D:(kc + 1) * D],
                              in_=wqt[kc * P:(kc + 1) * P, :])
        for kc in range(NKC):
            nc.sync.dma_start(out=wot_sb[:, kc * D:(kc + 1) * D],
                              in_=wot[kc * P:(kc + 1) * P, :])
        nc.sync.dma_start(out=bo_row[:], in_=bo_in[0:1, :])
        ones_row = const.tile([1, P], F32)
        nc.vector.memset(ones_row[:], 1.0)

        # ---- projections -------------------------------------------------
        # k^T [256, 2048] as 2 partition-blocks (kv head kv at block kv//2,
        # partition offset (kv%2)*64)
        kt_sb = work.tile([P, 2 * N], BF16, tag="kt")
        for m2 in range(2):
            for nb in range(4):
                ps = ppool.tile([P, 512], F32, tag="proj")
                for kc in range(NKC):
                    nc.tensor.matmul(
                        ps[:],
                        lhsT=wkt_sb[:, kc * KVD + m2 * P: kc * KVD + (m2 + 1) * P],
                        rhs=xt_sb[:, kc * N + nb * 512: kc * N + (nb + 1) * 512],
                        start=(kc == 0), stop=(kc == NKC - 1),
                    )
                nc.vector.tensor_copy(kt_sb[:, m2 * N + nb * 512: m2 * N + (nb + 1) * 512], ps[:])

        # v natural [keys, vdim], augmented: per key block 4 x [64 v | 1 one]
        # (a single ones column per kv head: the AV matmul only spends 65 of
        # 128 PE columns, halving its MAC power draw vs 64 duplicated ones)
        v_sb = work.tile([P, NKB * VW], BF16, tag="v")
        nc.vector.memset(v_sb[:], 1.0)  # ones columns survive the copies
        ones_bf = const.tile([1, P], BF16)
        nc.vector.memset(ones_bf[:], 1.0)
        for kb in range(NKB):
            ps = ppool.tile([P, KVD], F32, tag="proj")
            for kc in range(NKC):
                nc.tensor.matmul(
                    ps[:],
                    lhsT=xt_sb[:, kc * N + kb * P: kc * N + (kb + 1) * P],
                    rhs=wvt_sb[:, kc * KVD:(kc + 1) * KVD],
                    start=(kc == 0), stop=(kc == NKC - 1),
                )
            for kv in range(4):
                nc.vector.tensor_copy(
                    v_sb[:, kb * VW + kv * 65: kb * VW + kv * 65 + 64],
                    ps[:, kv * 64:(kv + 1) * 64])

        # q^T [1024, 1024] in device head order, 8 partition-block tiles
        qt = []
        for j in range(8):
            qt_j = work.tile([P, NQ], BF16, tag=f"qt{j}")
            qt.append(qt_j)
            for q2 in range(2):
                ps = ppool.tile([P, 512], F32, tag="proj")
                for kc in range(NKC):
                    nc.tensor.matmul(
                        ps[:],
                        lhsT=wqt_sb[:, kc * D + j * P: kc * D + (j + 1) * P],
                        rhs=xt_sb[:, kc * N + q2 * 512: kc * N + (q2 + 1) * 512],
                        start=(kc == 0), stop=(kc == NKC - 1),
                    )
                nc.vector.tensor_copy(qt_j[:, q2 * 512:(q2 + 1) * 512], ps[:])

        # bias partition-broadcast via PE outer product (emitted after the
        # projections so the PE queue never stalls on the bo DMA up front)
        bo_bc = const.tile([P, D], F32)
        for jh in range(2):
            bps = ppool.tile([P, 512], F32, tag="proj")
            nc.tensor.matmul(bps[:], lhsT=ones_row[:, 0:P],
                             rhs=bo_row[:, jh * 512:(jh + 1) * 512],
                             start=True, stop=True)
            nc.vector.tensor_copy(bo_bc[:, jh * 512:(jh + 1) * 512], bps[:])

        # hidden^T [1024, 1024] bf16, device head order (matches wot rows)
        hid = []
        for j in range(8):
            hid_j = work.tile([P, NQ], BF16, tag=f"hid{j}")
            hid.append(hid_j)

        # ---- attention + streamed o-proj ---------------------------------
        for qb in range(2):  # 512-query blocks
            for j in range(8):
                for o in (0, 64):
                    kv = 2 * (j // 4) + o // 64
                    av = ppool.tile([P, 512], F32, tag="proj")
                    for kb2 in range(NKB // 2):  # key blocks in fused pairs
                        st = stpool.tile([P, 1024], F32, tag="st")  # 2 banks
                        for u in range(2):
                            kb = 2 * kb2 + u
                            nc.tensor.matmul(
                                st[:, u * 512:(u + 1) * 512],
                                lhsT=kt_sb[o:o + 64, (j // 4) * N + kb * P: (j // 4) * N + (kb + 1) * P],
                                rhs=qt[j][o:o + 64, qb * 512:(qb + 1) * 512],
                                start=True, stop=True,
                            )
                        # one ACT pass over both key blocks amortizes the
                        # ~290ns ACTIVATE pipeline overhead
                        pt = ptpool.tile([P, 1024], BF16, tag="pt")
                        nc.scalar.activation(pt[:], st[:],
                                             mybir.ActivationFunctionType.Exp,
                                             scale=SCALE)
                        for u in range(2):
                            kb = 2 * kb2 + u
                            nc.tensor.matmul(
                                av[0:65, :],
                                lhsT=v_sb[:, kb * VW + kv * 65: kb * VW + kv * 65 + 65],
                                rhs=pt[:, u * 512:(u + 1) * 512],
                                start=(kb == 0), stop=(kb == NKB - 1),
                            )
                    # reciprocal of the single denominator row, then
                    # partition-broadcast over 64 lanes via K=1 PE outer
                    # product (bf16 stationary: 1 cycle/row)
                    den_r = smallp.tile([1, 512], F32, tag="denr")
                    nc.vector.reciprocal(den_r[:], av[64:65, :])
                    den_rb = smallp.tile([1, 512], BF16, tag="denb")
                    nc.vector.tensor_copy(den_rb[:], den_r[:])
                    den_ps = ppool.tile([64, 512], F32, tag="proj")
                    nc.tensor.matmul(den_ps[:], lhsT=ones_bf[:, 0:64],
                                     rhs=den_rb[:], start=True, stop=True)
                    den_sb = smallp.tile([64, 512], F32, tag="dens")
                    nc.vector.tensor_copy(den_sb[:], den_ps[:])
                    nc.vector.tensor_tensor(
                        out=hid[j][o:o + 64, qb * 512:(qb + 1) * 512],
                        in0=av[0:64, :], in1=den_sb[:],
                        op=mybir.AluOpType.mult,
                    )

            # o-proj + bias + output DMA for this query block
            for tb in range(4):
                ot = outp.tile([P, D], F32, tag="osb")
                for jh in range(2):
                    ps = ppool.tile([P, 512], F32, tag="proj")
                    for ic in range(8):
                        nc.tensor.matmul(
                            ps[:],
                            lhsT=hid[ic][:, qb * 512 + tb * P: qb * 512 + (tb + 1) * P],
                            rhs=wot_sb[:, ic * D + jh * 512: ic * D + (jh + 1) * 512],
                            start=(ic == 0), stop=(ic == 7),
                        )
                    nc.vector.tensor_tensor(
                        out=ot[:, jh * 512:(jh + 1) * 512],
                        in0=ps[:], in1=bo_bc[:, jh * 512:(jh + 1) * 512],
                        op=mybir.AluOpType.add,
                    )
                nc.sync.dma_start(
                    out=out_p[qb * 512 + tb * P: qb * 512 + (tb + 1) * P, :],
                    in_=ot[:])

    _split_excess_waits(nc)
    return nc


def make_in_maps(x, wq, wk, wv, wo, bo):
    bf = ml_dtypes.bfloat16
    # device q block j (0..7), offset o in {0,64}: head kv=2*(j//4)+o//64,
    # g=j%4; original wq row for (kv, g, lane l) = kv*256 + g*64 + l
    dperm = np.empty(D, np.int64)
    for j in range(8):
        for o in (0, 1):
            kv = 2 * (j // 4) + o
            g = j % 4
            base = j * 128 + o * 64
            dperm[base:base + 64] = np.arange(kv * 256 + g * 64, kv * 256 + g * 64 + 64)
    wqt_h = np.ascontiguousarray(wq[dperm].T).astype(bf)   # [1024, 1024 dev dims]
    wkt_h = np.ascontiguousarray(wk.T).astype(bf)          # [1024, 256]
    wvt_h = np.ascontiguousarray(wv.T).astype(bf)
    wot_h = np.ascontiguousarray(wo.T[dperm]).astype(bf)   # [1024 dev dims, 1024]
    bo_h = bo.astype(np.float32).reshape(1, D)
    in_maps = []
    for c in range(NCORES):
        b, qh = c // 2, c % 2
        xb = x[b]
        if qh:
            xb = np.concatenate([xb[NQ:], xb[:NQ]], axis=0)  # own queries first
        in_maps.append({
            "xt": np.ascontiguousarray(xb.T).astype(bf),
            "wqt": wqt_h,
            "wkt": wkt_h,
            "wvt": wvt_h,
            "wot": wot_h,
            "bo_in": bo_h,
        })
    return in_maps


_CACHED_NC = None


def kernel(x, wq, wk, wv, wo, bo, _trace=False, _trace_kwargs=None):
    global _CACHED_NC
    from concourse.bass_utils import run_bass_kernel_spmd

    if _CACHED_NC is None:
        _CACHED_NC = build_nc()
    nc = _CACHED_NC

    in_maps = make_in_maps(
        np.asarray(x, np.float32), np.asarray(wq, np.float32),
        np.asarray(wk, np.float32), np.asarray(wv, np.float32),
        np.asarray(wo, np.float32), np.asarray(bo, np.float32))

    res = run_bass_kernel_spmd(
        nc, in_maps, core_ids=list(range(NCORES)),
        trace=_trace, **(_trace_kwargs or {}))

    out = np.empty((B, N, D), np.float32)
    for c in range(NCORES):
        b, qh = c // 2, c % 2
        out[b, qh * NQ:(qh + 1) * NQ] = res.results[c]["out_p"]
    if _trace:
        kernel._last_results = res
    return out


# revision 17
# speedup vs baseline: 1.4851x; 1.4851x over previous
"""GQA attention kernel for 8 TRN2 NeuronCores — sequence-split edition.

Problem: x[4,2048,1024], 16 Q heads / 4 KV heads, head_dim 64 (torch-Linear
style projections, softmax(QK^T/8)V, output projection + bias).

Sharding: core c handles (batch b = c//2, query-half qh = c%2): it computes
the FULL output rows for its 1024 query tokens (all 16 heads + o-proj), so
there is NO collective — each core DMAs its own [1024, 1024] f32 slab out.
K/V are computed for all 2048 keys on both cores of a pair (cheap).

The host permutes each core's token order so its own queries come first
(attention is key-order invariant), which keeps the SPMD program uniform.
Q-head order is permuted on the host so each head's 64 q-dims sit at the
same partition offset as its KV head's k-dims (QK lhsT/rhs share a base
partition): device q block j (0..7), offset o in {0,64} holds head
(kv = 2*(j//4) + o//64, g = j%4); wo^T rows are permuted identically.

Inside: q^T/k^T in [dim, token] layout so QK needs no transposes; S^T tiles
[keys=128, queries=512] are exp'd on ACT straight from PSUM; V is augmented
with 64 ones columns so the AV matmul also produces the softmax denominators
on partitions 64:128; normalization = reciprocal_approx_fast (single fast
custom-DVE op) + one DVE multiply writing hid^T in bf16. O-proj + bias-add +
output DMA stream per 512-query block, overlapped with the next block's
attention (ACT exp is the critical engine; everything else hides under it).
"""

import sys
import numpy as np
from contextlib import ExitStack

sys.path.insert(0, "/opt/trn_rl_repo")

import ml_dtypes

from concourse import bass, tile, mybir


# ---------------------------------------------------------------------------
# This walrus build encodes at most 1-2 sync waits per instruction; the stock
# TileContext tail drain packs one wait per live proc onto a single Drain and
# fails codegen ("Too many sync wait commands"). Spread the waits over SP nop
# carriers instead.
def _patched_drain_and_barrier(self, tick_clock, wait_clock):
    from concourse.vector_clock import ScopedClock, VectorClock

    nc = self.nc
    gc = tick_clock.global_clock
    n = len(gc)
    for proc in range(n):
        t = gc[proc]
        if t <= 0:
            continue
        carrier = nc.sync.nop(nofuse=True)
        req = VectorClock([t if i == proc else 0 for i in range(n)])
        wait_clock.add_sem_waits(carrier.ins, ScopedClock({None: req}))
    nc.sync.drain()
    nc.all_engine_barrier()
    assert self.sems is not None
    popped = nc._tile_sem_poison_stack.pop()
    assert popped is self._sem_poison
    nc.clear_and_free_semaphores(list(self.sems.allocated().values()))
    nc.all_engine_barrier()


tile.TileContext._drain_and_barrier = _patched_drain_and_barrier


def _split_excess_waits(nc, max_waits=1):
    """Hoist all but one sync wait per instruction onto dedicated
    EventSemaphore carriers placed immediately before it on the same engine
    (same blocking semantics, one wait per encoded instruction)."""
    n_new = 0
    for bb in nc.main_func.blocks:
        il = list(bb.instructions)
        out = []
        changed = False
        for ins in il:
            si = ins.sync_info
            if si is not None:
                w = list(si.on_wait)
                if len(w) > max_waits:
                    for extra in w[max_waits:]:
                        ev = mybir.InstEventSemaphore(
                            name=f"{ins.name}-wsp{n_new}", engine=ins.engine)
                        n_new += 1
                        ev.sync_info = type(si)(on_wait=[extra], on_update=[])
                        nc.register_instruction(ev, overwrite=True)
                        out.append(ev)
                    si.on_wait = w[:max_waits]
                    changed = True
            out.append(ins)
        if changed:
            bb.instructions = out
# ---------------------------------------------------------------------------

B, N, D = 4, 2048, 1024
DH = 64          # head dim
NQ = 1024        # queries per core
NCORES = 8
P = 128
SCALE = DH ** -0.5
BF16 = mybir.dt.bfloat16
F32 = mybir.dt.float32

NKB = N // P     # 16 key blocks of 128
NKC = D // P     # 8 contraction chunks of 128
KVD = 256        # total kv dims
VW = 512         # v chunk width per key block: 4 x [64 v | 64 ones]


def build_nc(st_bufs=2, av_bufs=2, pt_bufs=4):
    nc = bass.Bass(target_bir_lowering=False, debug=False, num_devices=NCORES)

    xt = nc.declare_dram_parameter("xt", [D, N], BF16, isOutput=False)
    wqt = nc.declare_dram_parameter("wqt", [D, D], BF16, isOutput=False)
    wkt = nc.declare_dram_parameter("wkt", [D, KVD], BF16, isOutput=False)
    wvt = nc.declare_dram_parameter("wvt", [D, KVD], BF16, isOutput=False)
    wot = nc.declare_dram_parameter("wot", [D, D], BF16, isOutput=False)
    bo_in = nc.declare_dram_parameter("bo_in", [1, D], F32, isOutput=False)
    out_p = nc.declare_dram_parameter("out_p", [NQ, D], F32, isOutput=True)

    with tile.TileContext(nc) as tc, ExitStack() as ctx:
        const = ctx.enter_context(tc.tile_pool(name="const", bufs=1))
        work = ctx.enter_context(tc.tile_pool(name="work", bufs=1))
        # one shared 4-buf PSUM pool for proj/o-proj/AV tiles (deep av
        # rotation so block N+4's wait is always satisfied) + 2x 2-bank st
        ppool = ctx.enter_context(tc.tile_pool(name="ppool", bufs=4, space="PSUM"))
        stpool = ctx.enter_context(tc.tile_pool(name="stp", bufs=st_bufs, space="PSUM"))
        ptpool = ctx.enter_context(tc.tile_pool(name="ptp", bufs=pt_bufs))
        smallp = ctx.enter_context(tc.tile_pool(name="smallp", bufs=3))
        outp = ctx.enter_context(tc.tile_pool(name="outp", bufs=3))

        # ---- load inputs (K weights + x first: K-proj unblocks earliest) --
        xt_sb = const.tile([P, NKC * N], BF16)
        wkt_sb = const.tile([P, NKC * KVD], BF16)
        wvt_sb = const.tile([P, NKC * KVD], BF16)
        wqt_sb = const.tile([P, NKC * D], BF16)
        wot_sb = const.tile([P, NKC * D], BF16)
        bo_row = const.tile([1, D], F32)
        for kc in range(NKC):
            nc.sync.dma_start(out=wkt_sb[:, kc * KVD:(kc + 1) * KVD],
                              in_=wkt[kc * P:(kc + 1) * P, :])
        for kc in range(NKC):
            # x split across two DGE queues so the 4MB load halves in time
            eng = nc.gpsimd if kc % 2 else nc.sync
            eng.dma_start(out=xt_sb[:, kc * N:(kc + 1) * N],
                          in_=xt[kc * P:(kc + 1) * P, :])
        for kc in range(NKC):
            nc.sync.dma_start(out=wvt_sb[:, kc * KVD:(kc + 1) * KVD],
                              in_=wvt[kc * P:(kc + 1) * P, :])
        for kc in range(NKC):
            nc.sync.dma_start(out=wqt_sb[:, kc * D:(kc + 1) * D],
                              in_=wqt[kc * P:(kc + 1) * P, :])
        for kc in range(NKC):
            nc.sync.dma_start(out=wot_sb[:, kc * D:(kc + 1) * D],
                              in_=wot[kc * P:(kc + 1) * P, :])
        nc.sync.dma_start(out=bo_row[:], in_=bo_in[0:1, :])
        ones_row = const.tile([1, P], F32)
        nc.vector.memset(ones_row[:], 1.0)

        # ---- projections -------------------------------------------------
        # k^T [256, 2048] as 2 partition-blocks (kv head kv at block kv//2,
        # partition offset (kv%2)*64)
        kt_sb = work.tile([P, 2 * N], BF16, tag="kt")
        for m2 in range(2):
            for nb in range(4):
                ps = ppool.tile([P, 512], F32, tag="proj")
                for kc in range(NKC):
                    nc.tensor.matmul(
                        ps[:],
                        lhsT=wkt_sb[:, kc * KVD + m2 * P: kc * KVD + (m2 + 1) * P],
                        rhs=xt_sb[:, kc * N + nb * 512: kc * N + (nb + 1) * 512],
                        start=(kc == 0), stop=(kc == NKC - 1),
                    )
                nc.vector.tensor_copy(kt_sb[:, m2 * N + nb * 512: m2 * N + (nb + 1) * 512], ps[:])

        # v natural [keys, vdim], augmented: per key block 4 x [64 v | 64 ones]
        # (the duplicated ones columns make the AV matmul emit the softmax
        # denominators on partitions 64:128, lane-aligned with the numerators)
        v_sb = work.tile([P, NKB * VW], BF16, tag="v")
        nc.vector.memset(v_sb[:], 1.0)  # ones columns survive the copies
        for kb in range(NKB):
            ps = ppool.tile([P, KVD], F32, tag="proj")
            for kc in range(NKC):
                nc.tensor.matmul(
                    ps[:],
                    lhsT=xt_sb[:, kc * N + kb * P: kc * N + (kb + 1) * P],
                    rhs=wvt_sb[:, kc * KVD:(kc + 1) * KVD],
                    start=(kc == 0), stop=(kc == NKC - 1),
                )
            for kv in range(4):
                nc.vector.tensor_copy(
                    v_sb[:, kb * VW + kv * P: kb * VW + kv * P + 64],
                    ps[:, kv * 64:(kv + 1) * 64])

        # q^T [1024, 1024] in device head order, 8 partition-block tiles;
        # the projection of block j is emitted right before its first
        # attention pass (dilutes the dense PE burst that trips the power
        # throttle, and lets ACT start ~40us earlier)
        qt = []
        for j in range(8):
            qt_j = work.tile([P, NQ], BF16, tag=f"qt{j}")
            qt.append(qt_j)

        bo_bc = const.tile([P, D], F32)

        # hidden^T [1024, 1024] bf16, device head order (matches wot rows)
        hid = []
        for j in range(8):
            hid_j = work.tile([P, NQ], BF16, tag=f"hid{j}")
            hid.append(hid_j)

        # ---- attention + streamed o-proj ---------------------------------
        for qb in range(2):  # 512-query blocks
            for j in range(8):
                if qb == 0:
                    for q2 in range(2):
                        ps = ppool.tile([P, 512], F32, tag="proj")
                        for kc in range(NKC):
                            nc.tensor.matmul(
                                ps[:],
                                lhsT=wqt_sb[:, kc * D + j * P: kc * D + (j + 1) * P],
                                rhs=xt_sb[:, kc * N + q2 * 512: kc * N + (q2 + 1) * 512],
                                start=(kc == 0), stop=(kc == NKC - 1),
                            )
                        nc.vector.tensor_copy(qt[j][:, q2 * 512:(q2 + 1) * 512], ps[:])
                for o in (0, 64):
                    kv = 2 * (j // 4) + o // 64
                    av = ppool.tile([P, 512], F32, tag="proj")
                    for kb2 in range(NKB // 2):  # key blocks in fused pairs
                        st = stpool.tile([P, 1024], F32, tag="st")  # 2 banks
                        for u in range(2):
                            kb = 2 * kb2 + u
                            nc.tensor.matmul(
                                st[:, u * 512:(u + 1) * 512],
                                lhsT=kt_sb[o:o + 64, (j // 4) * N + kb * P: (j // 4) * N + (kb + 1) * P],
                                rhs=qt[j][o:o + 64, qb * 512:(qb + 1) * 512],
                                start=True, stop=True,
                            )
                        # one ACT pass over both key blocks amortizes the
                        # ~290ns ACTIVATE pipeline overhead
                        pt = ptpool.tile([P, 1024], BF16, tag="pt")
                        nc.scalar.activation(pt[:], st[:],
                                             mybir.ActivationFunctionType.Exp,
                                             scale=SCALE)
                        for u in range(2):
                            kb = 2 * kb2 + u
                            nc.tensor.matmul(
                                av[:],
                                lhsT=v_sb[:, kb * VW + kv * P: kb * VW + (kv + 1) * P],
                                rhs=pt[:, u * 512:(u + 1) * 512],
                                start=(kb == 0), stop=(kb == NKB - 1),
                            )
                    den = smallp.tile([64, 512], F32, tag="den")
                    nc.vector.reciprocal(den[:], av[64:128, :])
                    nc.vector.tensor_tensor(
                        out=hid[j][o:o + 64, qb * 512:(qb + 1) * 512],
                        in0=av[0:64, :], in1=den[:],
                        op=mybir.AluOpType.mult,
                    )

            if qb == 0:
                # bias partition-broadcast via PE outer product (needed from
                # the first o-proj; emitted late so the PE queue never stalls
                # on the bo DMA up front)
                for jh in range(2):
                    bps = ppool.tile([P, 512], F32, tag="proj")
                    nc.tensor.matmul(bps[:], lhsT=ones_row[:, 0:P],
                                     rhs=bo_row[:, jh * 512:(jh + 1) * 512],
                                     start=True, stop=True)
                    nc.vector.tensor_copy(bo_bc[:, jh * 512:(jh + 1) * 512], bps[:])

            # o-proj + bias + output DMA for this query block
            for tb in range(4):
                ot = outp.tile([P, D], F32, tag="osb")
                for jh in range(2):
                    ps = ppool.tile([P, 512], F32, tag="proj")
                    for ic in range(8):
                        nc.tensor.matmul(
                            ps[:],
                            lhsT=hid[ic][:, qb * 512 + tb * P: qb * 512 + (tb + 1) * P],
                            rhs=wot_sb[:, ic * D + jh * 512: ic * D + (jh + 1) * 512],
                            start=(ic == 0), stop=(ic == 7),
                        )
                    nc.vector.tensor_tensor(
                        out=ot[:, jh * 512:(jh + 1) * 512],
                        in0=ps[:], in1=bo_bc[:, jh * 512:(jh + 1) * 512],
                        op=mybir.AluOpType.add,
                    )
                nc.sync.dma_start(
                    out=out_p[qb * 512 + tb * P: qb * 512 + (tb + 1) * P, :],
                    in_=ot[:])

    _split_excess_waits(nc)
    return nc


def make_in_maps(x, wq, wk, wv, wo, bo):
    bf = ml_dtypes.bfloat16
    # device q block j (0..7), offset o in {0,64}: head kv=2*(j//4)+o//64,
    # g=j%4; original wq row for (kv, g, lane l) = kv*256 + g*64 + l
    dperm = np.empty(D, np.int64)
    for j in range(8):
        for o in (0, 1):
            kv = 2 * (j // 4) + o
            g = j % 4
            base = j * 128 + o * 64
            dperm[base:base + 64] = np.arange(kv * 256 + g * 64, kv * 256 + g * 64 + 64)
    wqt_h = np.ascontiguousarray(wq[dperm].T).astype(bf)   # [1024, 1024 dev dims]
    wkt_h = np.ascontiguousarray(wk.T).astype(bf)          # [1024, 256]
    wvt_h = np.ascontiguousarray(wv.T).astype(bf)
    wot_h = np.ascontiguousarray(wo.T[dperm]).astype(bf)   # [1024 dev dims, 1024]
    bo_h = bo.astype(np.float32).reshape(1, D)
    in_maps = []
    for c in range(NCORES):
        b, qh = c // 2, c % 2
        xb = x[b]
        if qh:
            xb = np.concatenate([xb[NQ:], xb[:NQ]], axis=0)  # own queries first
        in_maps.append({
            "xt": np.ascontiguousarray(xb.T).astype(bf),
            "wqt": wqt_h,
            "wkt": wkt_h,
            "wvt": wvt_h,
            "wot": wot_h,
            "bo_in": bo_h,
        })
    return in_maps


_CACHED_NC = None


def kernel(x, wq, wk, wv, wo, bo, _trace=False, _trace_kwargs=None):
    global _CACHED_NC
    from concourse.bass_utils import run_bass_kernel_spmd

    if _CACHED_NC is None:
        _CACHED_NC = build_nc()
    nc = _CACHED_NC

    in_maps = make_in_maps(
        np.asarray(x, np.float32), np.asarray(wq, np.float32),
        np.asarray(wk, np.float32), np.asarray(wv, np.float32),
        np.asarray(wo, np.float32), np.asarray(bo, np.float32))

    res = run_bass_kernel_spmd(
        nc, in_maps, core_ids=list(range(NCORES)),
        trace=_trace, **(_trace_kwargs or {}))

    out = np.empty((B, N, D), np.float32)
    for c in range(NCORES):
        b, qh = c // 2, c % 2
        out[b, qh * NQ:(qh + 1) * NQ] = res.results[c]["out_p"]
    if _trace:
        kernel._last_results = res
    return out


# revision 21
# speedup vs baseline: 1.4943x; 1.0062x over previous
"""GQA attention kernel for 8 TRN2 NeuronCores — sequence-split edition.

Problem: x[4,2048,1024], 16 Q heads / 4 KV heads, head_dim 64 (torch-Linear
style projections, softmax(QK^T/8)V, output projection + bias).

Sharding: core c handles (batch b = c//2, query-half qh = c%2): it computes
the FULL output rows for its 1024 query tokens (all 16 heads + o-proj), so
there is NO collective — each core DMAs its own [1024, 1024] f32 slab out.
K/V are computed for all 2048 keys on both cores of a pair (cheap).

The host permutes each core's token order so its own queries come first
(attention is key-order invariant), which keeps the SPMD program uniform.
Q-head order is permuted on the host so each head's 64 q-dims sit at the
same partition offset as its KV head's k-dims (QK lhsT/rhs share a base
partition): device q block j (0..7), offset o in {0,64} holds head
(kv = 2*(j//4) + o//64, g = j%4); wo^T rows are permuted identically.

Inside: q^T/k^T in [dim, token] layout so QK needs no transposes; S^T tiles
[keys=128, queries=512] are exp'd on ACT straight from PSUM; V is augmented
with 64 ones columns so the AV matmul also produces the softmax denominators
on partitions 64:128; normalization = reciprocal_approx_fast (single fast
custom-DVE op) + one DVE multiply writing hid^T in bf16. O-proj + bias-add +
output DMA stream per 512-query block, overlapped with the next block's
attention (ACT exp is the critical engine; everything else hides under it).
"""

import sys
import numpy as np
from contextlib import ExitStack

sys.path.insert(0, "/opt/trn_rl_repo")

import ml_dtypes

from concourse import bass, tile, mybir


# ---------------------------------------------------------------------------
# This walrus build encodes at most 1-2 sync waits per instruction; the stock
# TileContext tail drain packs one wait per live proc onto a single Drain and
# fails codegen ("Too many sync wait commands"). Spread the waits over SP nop
# carriers instead.
def _patched_drain_and_barrier(self, tick_clock, wait_clock):
    from concourse.vector_clock import ScopedClock, VectorClock

    nc = self.nc
    gc = tick_clock.global_clock
    n = len(gc)
    for proc in range(n):
        t = gc[proc]
        if t <= 0:
            continue
        carrier = nc.sync.nop(nofuse=True)
        req = VectorClock([t if i == proc else 0 for i in range(n)])
        wait_clock.add_sem_waits(carrier.ins, ScopedClock({None: req}))
    nc.sync.drain()
    nc.all_engine_barrier()
    assert self.sems is not None
    popped = nc._tile_sem_poison_stack.pop()
    assert popped is self._sem_poison
    nc.clear_and_free_semaphores(list(self.sems.allocated().values()))
    nc.all_engine_barrier()


tile.TileContext._drain_and_barrier = _patched_drain_and_barrier


def _split_excess_waits(nc, max_waits=1):
    """Hoist all but one sync wait per instruction onto dedicated
    EventSemaphore carriers placed immediately before it on the same engine
    (same blocking semantics, one wait per encoded instruction)."""
    n_new = 0
    for bb in nc.main_func.blocks:
        il = list(bb.instructions)
        out = []
        changed = False
        for ins in il:
            si = ins.sync_info
            if si is not None:
                w = list(si.on_wait)
                if len(w) > max_waits:
                    for extra in w[max_waits:]:
                        ev = mybir.InstEventSemaphore(
                            name=f"{ins.name}-wsp{n_new}", engine=ins.engine)
                        n_new += 1
                        ev.sync_info = type(si)(on_wait=[extra], on_update=[])
                        nc.register_instruction(ev, overwrite=True)
                        out.append(ev)
                    si.on_wait = w[:max_waits]
                    changed = True
            out.append(ins)
        if changed:
            bb.instructions = out
# ---------------------------------------------------------------------------

B, N, D = 4, 2048, 1024
DH = 64          # head dim
NQ = 1024        # queries per core
NCORES = 8
P = 128
SCALE = DH ** -0.5
BF16 = mybir.dt.bfloat16
F32 = mybir.dt.float32

NKB = N // P     # 16 key blocks of 128
NKC = D // P     # 8 contraction chunks of 128
KVD = 256        # total kv dims
VW = 512         # v chunk width per key block: 4 x [64 v | 64 ones]


def build_nc(st_bufs=2, av_bufs=2, pt_bufs=4):
    nc = bass.Bass(target_bir_lowering=False, debug=False, num_devices=NCORES)

    xt = nc.declare_dram_parameter("xt", [D, N], BF16, isOutput=False)
    wqt = nc.declare_dram_parameter("wqt", [D, D], BF16, isOutput=False)
    wkt = nc.declare_dram_parameter("wkt", [D, KVD], BF16, isOutput=False)
    wvt = nc.declare_dram_parameter("wvt", [D, KVD], BF16, isOutput=False)
    wot = nc.declare_dram_parameter("wot", [D, D], BF16, isOutput=False)
    bo_in = nc.declare_dram_parameter("bo_in", [1, D], F32, isOutput=False)
    out_p = nc.declare_dram_parameter("out_p", [NQ, D], F32, isOutput=True)

    with tile.TileContext(nc) as tc, ExitStack() as ctx:
        const = ctx.enter_context(tc.tile_pool(name="const", bufs=1))
        work = ctx.enter_context(tc.tile_pool(name="work", bufs=1))
        # one shared 4-buf PSUM pool for proj/o-proj/AV tiles (deep av
        # rotation so block N+4's wait is always satisfied) + 2x 2-bank st
        ppool = ctx.enter_context(tc.tile_pool(name="ppool", bufs=4, space="PSUM"))
        stpool = ctx.enter_context(tc.tile_pool(name="stp", bufs=st_bufs, space="PSUM"))
        ptpool = ctx.enter_context(tc.tile_pool(name="ptp", bufs=pt_bufs))
        smallp = ctx.enter_context(tc.tile_pool(name="smallp", bufs=3))
        outp = ctx.enter_context(tc.tile_pool(name="outp", bufs=3))

        # ---- load inputs (K weights + x first: K-proj unblocks earliest) --
        xt_sb = const.tile([P, NKC * N], BF16)
        wkt_sb = const.tile([P, NKC * KVD], BF16)
        wvt_sb = const.tile([P, NKC * KVD], BF16)
        wqt_sb = const.tile([P, NKC * D], BF16)
        wot_sb = const.tile([P, NKC * D], BF16)
        bo_row = const.tile([1, D], F32)
        for kc in range(NKC):
            nc.sync.dma_start(out=wkt_sb[:, kc * KVD:(kc + 1) * KVD],
                              in_=wkt[kc * P:(kc + 1) * P, :])
        for kc in range(NKC):
            # x striped across three DGE queues to cut the 4MB load time
            eng = (nc.gpsimd, nc.scalar, nc.sync)[kc % 3]
            eng.dma_start(out=xt_sb[:, kc * N:(kc + 1) * N],
                          in_=xt[kc * P:(kc + 1) * P, :])
        for kc in range(NKC):
            nc.sync.dma_start(out=wvt_sb[:, kc * KVD:(kc + 1) * KVD],
                              in_=wvt[kc * P:(kc + 1) * P, :])
        for kc in range(NKC):
            nc.sync.dma_start(out=wqt_sb[:, kc * D:(kc + 1) * D],
                              in_=wqt[kc * P:(kc + 1) * P, :])
        for kc in range(NKC):
            nc.sync.dma_start(out=wot_sb[:, kc * D:(kc + 1) * D],
                              in_=wot[kc * P:(kc + 1) * P, :])
        nc.sync.dma_start(out=bo_row[:], in_=bo_in[0:1, :])
        ones_row = const.tile([1, P], F32)
        nc.vector.memset(ones_row[:], 1.0)

        # ---- projections -------------------------------------------------
        # k^T [256, 2048] as 2 partition-blocks (kv head kv at block kv//2,
        # partition offset (kv%2)*64)
        kt_sb = work.tile([P, 2 * N], BF16, tag="kt")
        for m2 in range(2):
            for nb in range(4):
                ps = ppool.tile([P, 512], F32, tag="proj")
                for kc in range(NKC):
                    nc.tensor.matmul(
                        ps[:],
                        lhsT=wkt_sb[:, kc * KVD + m2 * P: kc * KVD + (m2 + 1) * P],
                        rhs=xt_sb[:, kc * N + nb * 512: kc * N + (nb + 1) * 512],
                        start=(kc == 0), stop=(kc == NKC - 1),
                    )
                nc.vector.tensor_copy(kt_sb[:, m2 * N + nb * 512: m2 * N + (nb + 1) * 512], ps[:])

        # v natural [keys, vdim], augmented: per key block 4 x [64 v | 64 ones]
        # (the duplicated ones columns make the AV matmul emit the softmax
        # denominators on partitions 64:128, lane-aligned with the numerators)
        v_sb = work.tile([P, NKB * VW], BF16, tag="v")
        nc.vector.memset(v_sb[:], 1.0)  # ones columns survive the copies
        for kb in range(NKB):
            ps = ppool.tile([P, KVD], F32, tag="proj")
            for kc in range(NKC):
                nc.tensor.matmul(
                    ps[:],
                    lhsT=xt_sb[:, kc * N + kb * P: kc * N + (kb + 1) * P],
                    rhs=wvt_sb[:, kc * KVD:(kc + 1) * KVD],
                    start=(kc == 0), stop=(kc == NKC - 1),
                )
            for kv in range(4):
                nc.vector.tensor_copy(
                    v_sb[:, kb * VW + kv * P: kb * VW + kv * P + 64],
                    ps[:, kv * 64:(kv + 1) * 64])

        # q^T [1024, 1024] in device head order, 8 partition-block tiles;
        # the projection of block j is emitted right before its first
        # attention pass (dilutes the dense PE burst that trips the power
        # throttle, and lets ACT start ~40us earlier)
        qt = []
        for j in range(8):
            qt_j = work.tile([P, NQ], BF16, tag=f"qt{j}")
            qt.append(qt_j)

        bo_bc = const.tile([P, D], F32)

        # hidden^T [1024, 1024] bf16, device head order (matches wot rows)
        hid = []
        for j in range(8):
            hid_j = work.tile([P, NQ], BF16, tag=f"hid{j}")
            hid.append(hid_j)

        def oproj_tile(src_qb, tb):
            # o-proj + bias + output DMA for one 128-token slice
            ot = outp.tile([P, D], F32, tag="osb", name="ot")
            for jh in range(2):
                ps = ppool.tile([P, 512], F32, tag="proj", name="ps")
                for ic in range(8):
                    nc.tensor.matmul(
                        ps[:],
                        lhsT=hid[ic][:, src_qb * 512 + tb * P: src_qb * 512 + (tb + 1) * P],
                        rhs=wot_sb[:, ic * D + jh * 512: ic * D + (jh + 1) * 512],
                        start=(ic == 0), stop=(ic == 7),
                    )
                nc.vector.tensor_tensor(
                    out=ot[:, jh * 512:(jh + 1) * 512],
                    in0=ps[:], in1=bo_bc[:, jh * 512:(jh + 1) * 512],
                    op=mybir.AluOpType.add,
                )
            nc.sync.dma_start(
                out=out_p[src_qb * 512 + tb * P: src_qb * 512 + (tb + 1) * P, :],
                in_=ot[:])

        # ---- attention + streamed o-proj ---------------------------------
        for qb in range(2):  # 512-query blocks
            for j in range(8):
                if qb == 0:
                    for q2 in range(2):
                        ps = ppool.tile([P, 512], F32, tag="proj")
                        for kc in range(NKC):
                            nc.tensor.matmul(
                                ps[:],
                                lhsT=wqt_sb[:, kc * D + j * P: kc * D + (j + 1) * P],
                                rhs=xt_sb[:, kc * N + q2 * 512: kc * N + (q2 + 1) * 512],
                                start=(kc == 0), stop=(kc == NKC - 1),
                            )
                        nc.vector.tensor_copy(qt[j][:, q2 * 512:(q2 + 1) * 512], ps[:])
                for o in (0, 64):
                    kv = 2 * (j // 4) + o // 64
                    av = ppool.tile([P, 512], F32, tag="proj")

                    def qk_exp(kb2):
                        st = stpool.tile([P, 1024], F32, tag="st", name="st")
                        for u in range(2):
                            kb = 2 * kb2 + u
                            nc.tensor.matmul(
                                st[:, u * 512:(u + 1) * 512],
                                lhsT=kt_sb[o:o + 64, (j // 4) * N + kb * P: (j // 4) * N + (kb + 1) * P],
                                rhs=qt[j][o:o + 64, qb * 512:(qb + 1) * 512],
                                start=True, stop=True,
                            )
                        # one ACT pass over both key blocks amortizes the
                        # ~290ns ACTIVATE pipeline overhead
                        pt = ptpool.tile([P, 1024], BF16, tag="pt", name="pt")
                        nc.scalar.activation(pt[:], st[:],
                                             mybir.ActivationFunctionType.Exp,
                                             scale=SCALE)
                        return pt

                    def av_mm(kb2, pt):
                        for u in range(2):
                            kb = 2 * kb2 + u
                            nc.tensor.matmul(
                                av[:],
                                lhsT=v_sb[:, kb * VW + kv * P: kb * VW + (kv + 1) * P],
                                rhs=pt[:, u * 512:(u + 1) * 512],
                                start=(kb == 0), stop=(kb == NKB - 1),
                            )

                    # software-pipelined: AV of pair p is emitted after QK of
                    # pair p+1, so the PE never head-of-line blocks on the exp
                    prev_pt = qk_exp(0)
                    for kb2 in range(1, NKB // 2):
                        cur_pt = qk_exp(kb2)
                        av_mm(kb2 - 1, prev_pt)
                        prev_pt = cur_pt
                    av_mm(NKB // 2 - 1, prev_pt)

                    den = smallp.tile([64, 512], F32, tag="den")
                    nc.vector.reciprocal(den[:], av[64:128, :])
                    nc.vector.tensor_tensor(
                        out=hid[j][o:o + 64, qb * 512:(qb + 1) * 512],
                        in0=av[0:64, :], in1=den[:],
                        op=mybir.AluOpType.mult,
                    )

                if qb == 1 and j < 4:
                    # o-proj of query block 0 interleaved into qb1's
                    # attention (fills the PE slack under the ACT-paced
                    # blocks instead of bursting at the qb boundary)
                    oproj_tile(0, j)

            if qb == 0:
                # bias partition-broadcast via PE outer product (needed from
                # the first o-proj; emitted late so the PE queue never stalls
                # on the bo DMA up front)
                for jh in range(2):
                    bps = ppool.tile([P, 512], F32, tag="proj")
                    nc.tensor.matmul(bps[:], lhsT=ones_row[:, 0:P],
                                     rhs=bo_row[:, jh * 512:(jh + 1) * 512],
                                     start=True, stop=True)
                    nc.vector.tensor_copy(bo_bc[:, jh * 512:(jh + 1) * 512], bps[:])
            else:
                for tb in range(4):
                    oproj_tile(1, tb)

    _split_excess_waits(nc)
    return nc


def make_in_maps(x, wq, wk, wv, wo, bo):
    bf = ml_dtypes.bfloat16
    # device q block j (0..7), offset o in {0,64}: head kv=2*(j//4)+o//64,
    # g=j%4; original wq row for (kv, g, lane l) = kv*256 + g*64 + l
    dperm = np.empty(D, np.int64)
    for j in range(8):
        for o in (0, 1):
            kv = 2 * (j // 4) + o
            g = j % 4
            base = j * 128 + o * 64
            dperm[base:base + 64] = np.arange(kv * 256 + g * 64, kv * 256 + g * 64 + 64)
    wqt_h = np.ascontiguousarray(wq[dperm].T).astype(bf)   # [1024, 1024 dev dims]
    wkt_h = np.ascontiguousarray(wk.T).astype(bf)          # [1024, 256]
    wvt_h = np.ascontiguousarray(wv.T).astype(bf)
    wot_h = np.ascontiguousarray(wo.T[dperm]).astype(bf)   # [1024 dev dims, 1024]
    bo_h = bo.astype(np.float32).reshape(1, D)
    in_maps = []
    for c in range(NCORES):
        b, qh = c // 2, c % 2
        xb = x[b]
        if qh:
            xb = np.concatenate([xb[NQ:], xb[:NQ]], axis=0)  # own queries first
        in_maps.append({
            "xt": np.ascontiguousarray(xb.T).astype(bf),
            "wqt": wqt_h,
            "wkt": wkt_h,
            "wvt": wvt_h,
            "wot": wot_h,
            "bo_in": bo_h,
        })
    return in_maps


_CACHED_NC = None


def kernel(x, wq, wk, wv, wo, bo, _trace=False, _trace_kwargs=None):
    global _CACHED_NC
    from concourse.bass_utils import run_bass_kernel_spmd

    if _CACHED_NC is None:
        _CACHED_NC = build_nc()
    nc = _CACHED_NC

    in_maps = make_in_maps(
        np.asarray(x, np.float32), np.asarray(wq, np.float32),
        np.asarray(wk, np.float32), np.asarray(wv, np.float32),
        np.asarray(wo, np.float32), np.asarray(bo, np.float32))

    res = run_bass_kernel_spmd(
        nc, in_maps, core_ids=list(range(NCORES)),
        trace=_trace, **(_trace_kwargs or {}))

    out = np.empty((B, N, D), np.float32)
    for c in range(NCORES):
        b, qh = c // 2, c % 2
        out[b, qh * NQ:(qh + 1) * NQ] = res.results[c]["out_p"]
    if _trace:
        kernel._last_results = res
    return out
